# revision 5
# baseline (speedup 1.0000x reference)
"""Trainium2 Bass kernel for nn_CoordinationMemory (scatter_memory).

Per-row op: gather cur_h = memory[r, idx_r]; h = x_r @ W_in + cur_h @ W_h + b;
LayerNorm; tanh; scatter back into a full copy of memory.

Sharding: N=4096 rows split across 8 cores (512 rows each). Weights are
replicated. Each core:
  - bulk-copies its 64 MB memory shard DRAM->DRAM (input -> output),
  - gathers its 512 current rows via indirect DMA (flat indices precomputed
    on host as r*L_V + idx_r),
  - computes the MLP + LN + tanh on PE/DVE/ACT,
  - scatters the 512 updated rows into the output after the copy chunk
    covering those rows has landed.
"""

import numpy as np

import concourse.tile as tile
from concourse import bacc, bass, mybir
from concourse.bass_utils import run_bass_kernel_spmd
from concourse.masks import make_identity

N, L_V, H, D = 4096, 128, 256, 256
NCORES = 8
NS = N // NCORES            # rows per core = 512
P = 128                     # partitions
RT = NS // P                # row-tiles per core = 4
KC_IN = (3 * D) // P        # K chunks for W_in = 6
KC_H = H // P               # K chunks for W_h = 2
ROWS_FLAT = NS * L_V        # flattened memory rows per core = 65536
COPY_CHUNKS = RT            # one copy chunk per row-tile (aligned ranges)
LN_EPS = 1e-5

_CACHE: dict = {}
LAST_RESULT = None          # test harness reads exec_time_ns from here


def _build_bass() -> bass.Bass:
    f32 = mybir.dt.float32
    i32 = mybir.dt.int32
    nc = bacc.Bacc(None)

    mem = nc.declare_dram_parameter("mem", [ROWS_FLAT, H], f32, isOutput=False)
    xT = nc.declare_dram_parameter("xT", [3 * D, NS], f32, isOutput=False)
    idx = nc.declare_dram_parameter("idx", [NS, 1], i32, isOutput=False)
    w_in = nc.declare_dram_parameter("w_in", [3 * D, H], f32, isOutput=False)
    w_h = nc.declare_dram_parameter("w_h", [H, H], f32, isOutput=False)
    # vecs rows: 0 = b_in + b_h, 1 = gamma, 2 = beta
    vecs = nc.declare_dram_parameter("vecs", [3, H], f32, isOutput=False)
    out = nc.declare_dram_parameter("out", [ROWS_FLAT, H], f32, isOutput=True)

    with tile.TileContext(nc) as tc:
        with (
            tc.tile_pool(name="const", bufs=1) as const,
            tc.tile_pool(name="work", bufs=3) as work,
            tc.tile_pool(name="psum", bufs=2, space="PSUM") as psum,
        ):
            ident = const.tile([P, P], f32)
            make_identity(nc, ident[:])

            w_in_sb = const.tile([P, KC_IN, H], f32)
            nc.sync.dma_start(
                out=w_in_sb[:], in_=w_in[:].rearrange("(k p) n -> p k n", p=P)
            )
            w_h_sb = const.tile([P, KC_H, H], f32)
            nc.sync.dma_start(
                out=w_h_sb[:], in_=w_h[:].rearrange("(k p) n -> p k n", p=P)
            )
            xT_sb = const.tile([P, KC_IN, NS], f32)
            nc.sync.dma_start(
                out=xT_sb[:], in_=xT[:].rearrange("(k p) n -> p k n", p=P)
            )

            vec_ap = vecs[:]
            vec_bcast = bass.AP(
                tensor=vec_ap.tensor,
                offset=vec_ap.offset,
                ap=[[0, P]] + list(vec_ap.ap),
            )
            vec_sb = const.tile([P, 3, H], f32)
            nc.gpsimd.dma_start(out=vec_sb[:], in_=vec_bcast)

            eps_sb = const.tile([P, 1], f32)
            nc.vector.memset(eps_sb[:], LN_EPS)

            # Bulk copy of the memory shard, chunked so chunk t covers
            # exactly the flat rows row-tile t scatters into.
            chunk = ROWS_FLAT // COPY_CHUNKS
            copy_insts = []
            for c in range(COPY_CHUNKS):
                ci = nc.sync.dma_start(
                    out=out[c * chunk : (c + 1) * chunk, :],
                    in_=mem[c * chunk : (c + 1) * chunk, :],
                )
                copy_insts.append(ci)

            for t in range(RT):
                idx_sb = work.tile([P, 1], i32)
                nc.sync.dma_start(out=idx_sb[:], in_=idx[t * P : (t + 1) * P, :])

                curh = work.tile([P, H], f32)
                nc.gpsimd.indirect_dma_start(
                    out=curh[:],
                    out_offset=None,
                    in_=mem[:],
                    in_offset=bass.IndirectOffsetOnAxis(ap=idx_sb[:, :1], axis=0),
                )

                # cur_h^T (K on partitions) for the W_h matmul
                curhT = work.tile([P, KC_H, P], f32)
                for k in range(KC_H):
                    pt = psum.tile([P, P], f32)
                    nc.tensor.transpose(
                        out=pt[:], in_=curh[:, k * P : (k + 1) * P], identity=ident[:]
                    )
                    nc.vector.tensor_copy(out=curhT[:, k, :], in_=pt[:])

                ph = psum.tile([P, H], f32)
                for k in range(KC_IN):
                    nc.tensor.matmul(
                        out=ph[:],
                        lhsT=xT_sb[:, k, t * P : (t + 1) * P],
                        rhs=w_in_sb[:, k, :],
                        start=(k == 0),
                        stop=False,
                    )
                for k in range(KC_H):
                    nc.tensor.matmul(
                        out=ph[:],
                        lhsT=curhT[:, k, :],
                        rhs=w_h_sb[:, k, :],
                        start=False,
                        stop=(k == KC_H - 1),
                    )

                h_sb = work.tile([P, H], f32)
                nc.vector.tensor_add(out=h_sb[:], in0=ph[:], in1=vec_sb[:, 0, :])

                stats = work.tile([P, 6], f32)
                nc.vector.bn_stats(out=stats[:], in_=h_sb[:])
                mv = work.tile([P, 2], f32)
                nc.vector.bn_aggr(out=mv[:], in_=stats[:])
                # mv[:,1] = 1/sqrt(var + eps)
                nc.scalar.activation(
                    out=mv[:, 1:2],
                    in_=mv[:, 1:2],
                    func=mybir.ActivationFunctionType.Sqrt,
                    bias=eps_sb[:],
                    scale=1.0,
                )
                nc.vector.reciprocal(out=mv[:, 1:2], in_=mv[:, 1:2])
                # h = (h - mean) * rstd
                nc.vector.tensor_scalar(
                    out=h_sb[:],
                    in0=h_sb[:],
                    scalar1=mv[:, 0:1],
                    scalar2=mv[:, 1:2],
                    op0=mybir.AluOpType.subtract,
                    op1=mybir.AluOpType.mult,
                )
                nc.vector.tensor_mul(h_sb[:], h_sb[:], vec_sb[:, 1, :])
                nc.vector.tensor_add(out=h_sb[:], in0=h_sb[:], in1=vec_sb[:, 2, :])
                nc.scalar.activation(
                    out=h_sb[:],
                    in_=h_sb[:],
                    func=mybir.ActivationFunctionType.Tanh,
                )

                sc = nc.gpsimd.indirect_dma_start(
                    out=out[:],
                    out_offset=bass.IndirectOffsetOnAxis(ap=idx_sb[:, :1], axis=0),
                    in_=h_sb[:],
                    in_offset=None,
                )
                # Row-tile t scatters only into flat rows [t*chunk, (t+1)*chunk):
                # order after the matching copy chunk.
                tile.add_dep_helper(
                    sc.ins, copy_insts[t].ins, sync=True,
                    reason="scatter after bulk copy of same range",
                )

    nc.finalize()
    return nc


def _prepare_in_maps(inputs: dict) -> list[dict]:
    memory = np.ascontiguousarray(np.asarray(inputs["memory"], dtype=np.float32))
    veh_idx = np.asarray(inputs["veh_idx"]).astype(np.int64)
    veh = np.asarray(inputs["veh_repr"], dtype=np.float32).reshape(N, D)
    cust = np.asarray(inputs["cust_repr"], dtype=np.float32).reshape(N, D)
    edge = np.asarray(inputs["edge_emb"], dtype=np.float32).reshape(N, D)
    w_in = np.ascontiguousarray(np.asarray(inputs["W_in"], dtype=np.float32))
    b_in = np.asarray(inputs["b_in"], dtype=np.float32)
    w_h = np.ascontiguousarray(np.asarray(inputs["W_h"], dtype=np.float32))
    b_h = np.asarray(inputs["b_h"], dtype=np.float32)
    gamma = np.asarray(inputs["gamma"], dtype=np.float32)
    beta = np.asarray(inputs["beta"], dtype=np.float32)

    x = np.concatenate([veh, cust, edge], axis=1)  # [N, 3D]
    vecs = np.ascontiguousarray(np.stack([b_in + b_h, gamma, beta]))  # [3, H]
    flat_idx = (
        np.arange(N, dtype=np.int64) % NS * L_V + veh_idx[:, 0]
    ).astype(np.int32)

    in_maps = []
    for c in range(NCORES):
        rows = slice(c * NS, (c + 1) * NS)
        in_maps.append(
            {
                "mem": memory[rows].reshape(ROWS_FLAT, H),
                "xT": np.ascontiguousarray(x[rows].T),
                "idx": np.ascontiguousarray(flat_idx[rows].reshape(NS, 1)),
                "w_in": w_in,
                "w_h": w_h,
                "vecs": vecs,
            }
        )
    return in_maps


def get_nc() -> bass.Bass:
    if "nc" not in _CACHE:
        _CACHE["nc"] = _build_bass()
    return _CACHE["nc"]


def kernel(**inputs: np.ndarray) -> np.ndarray:
    nc = get_nc()
    in_maps = _prepare_in_maps(inputs)

    global LAST_RESULT
    LAST_RESULT = run_bass_kernel_spmd(nc, in_maps, list(range(NCORES)))
    res = LAST_RESULT.results
    return np.concatenate(
        [res[c]["out"].reshape(NS, L_V, H) for c in range(NCORES)], axis=0
    )


# revision 7
# speedup vs baseline: 1.0028x; 1.0028x over previous
"""Trainium2 Bass kernel for nn_CoordinationMemory (scatter_memory).

Per-row op: gather cur_h = memory[r, idx_r]; h = x_r @ W_in + cur_h @ W_h + b;
LayerNorm; tanh; scatter back into a full copy of memory.

Sharding: N=4096 rows split across 8 cores (512 rows each); weights
replicated. Per core the dominant cost is streaming its 64 MB memory shard
input->output through DMA. The output is declared as 4 chunk tensors
(one per 128-row tile) so each scatter depends only on its own chunk's
bulk copy — otherwise conservative whole-tensor DRAM dependency tracking
serializes every scatter (and everything queued behind it on the gpsimd
engine) after the whole copy. Gathers are issued up-front for the same
reason. The copy is split across both HWDGE rings (sync + scalar).
"""

import numpy as np

import concourse.tile as tile
from concourse import bacc, bass, mybir
from concourse.bass_utils import run_bass_kernel_spmd
from concourse.masks import make_identity

N, L_V, H, D = 4096, 128, 256, 256
NCORES = 8
NS = N // NCORES            # rows per core = 512
P = 128                     # partitions
RT = NS // P                # row-tiles per core = 4
KC_IN = (3 * D) // P        # K chunks for W_in = 6
KC_H = H // P               # K chunks for W_h = 2
ROWS_FLAT = NS * L_V        # flattened memory rows per core = 65536
CHUNK = ROWS_FLAT // RT     # flat rows per output chunk = 16384
LN_EPS = 1e-5

_CACHE: dict = {}
LAST_RESULT = None          # test harness reads exec_time_ns from here


def _build_bass() -> bass.Bass:
    f32 = mybir.dt.float32
    i32 = mybir.dt.int32
    nc = bacc.Bacc(None)

    mem = nc.declare_dram_parameter("mem", [ROWS_FLAT, H], f32, isOutput=False)
    xT = nc.declare_dram_parameter("xT", [3 * D, NS], f32, isOutput=False)
    idx = nc.declare_dram_parameter("idx", [NS, 2], i32, isOutput=False)
    w_in = nc.declare_dram_parameter("w_in", [3 * D, H], f32, isOutput=False)
    w_h = nc.declare_dram_parameter("w_h", [H, H], f32, isOutput=False)
    # vecs rows: 0 = b_in + b_h, 1 = gamma, 2 = beta
    vecs = nc.declare_dram_parameter("vecs", [3, H], f32, isOutput=False)
    outs = [
        nc.declare_dram_parameter(f"out{t}", [CHUNK, H], f32, isOutput=True)
        for t in range(RT)
    ]

    with tile.TileContext(nc) as tc:
        with (
            tc.tile_pool(name="const", bufs=1) as const,
            tc.tile_pool(name="work", bufs=4) as work,
            tc.tile_pool(name="psum", bufs=2, space="PSUM") as psum,
        ):
            # Index loads + bulk copy first so DMA rings saturate immediately.
            idx_sbs = []
            for t in range(RT):
                idx_sb = const.tile([P, 2], i32, tag=f"idx{t}")
                nc.gpsimd.dma_start(out=idx_sb[:], in_=idx[t * P : (t + 1) * P, :])
                idx_sbs.append(idx_sb)

            half = CHUNK // 2
            copy_insts = []  # per row-tile: list of copy instructions
            for t in range(RT):
                c0 = nc.sync.dma_start(
                    out=outs[t][:half, :],
                    in_=mem[t * CHUNK : t * CHUNK + half, :],
                )
                c1 = nc.scalar.dma_start(
                    out=outs[t][half:, :],
                    in_=mem[t * CHUNK + half : (t + 1) * CHUNK, :],
                )
                copy_insts.append([c0, c1])

            # Gathers up-front (gpsimd queue) so no scatter wait blocks them.
            curhs = []
            for t in range(RT):
                curh = work.tile([P, H], f32, tag=f"curh{t}")
                nc.gpsimd.indirect_dma_start(
                    out=curh[:],
                    out_offset=None,
                    in_=mem[:],
                    in_offset=bass.IndirectOffsetOnAxis(ap=idx_sbs[t][:, 0:1], axis=0),
                )
                curhs.append(curh)

            ident = const.tile([P, P], f32)
            make_identity(nc, ident[:])

            w_in_sb = const.tile([P, KC_IN, H], f32)
            nc.sync.dma_start(
                out=w_in_sb[:], in_=w_in[:].rearrange("(k p) n -> p k n", p=P)
            )
            w_h_sb = const.tile([P, KC_H, H], f32)
            nc.sync.dma_start(
                out=w_h_sb[:], in_=w_h[:].rearrange("(k p) n -> p k n", p=P)
            )
            xT_sb = const.tile([P, KC_IN, NS], f32)
            nc.sync.dma_start(
                out=xT_sb[:], in_=xT[:].rearrange("(k p) n -> p k n", p=P)
            )

            vec_ap = vecs[:]
            vec_bcast = bass.AP(
                tensor=vec_ap.tensor,
                offset=vec_ap.offset,
                ap=[[0, P]] + list(vec_ap.ap),
            )
            vec_sb = const.tile([P, 3, H], f32)
            nc.gpsimd.dma_start(out=vec_sb[:], in_=vec_bcast)

            eps_sb = const.tile([P, 1], f32)
            nc.vector.memset(eps_sb[:], LN_EPS)

            for t in range(RT):
                curh = curhs[t]
                # cur_h^T (K on partitions) for the W_h matmul
                curhT = work.tile([P, KC_H, P], f32)
                for k in range(KC_H):
                    pt = psum.tile([P, P], f32)
                    nc.tensor.transpose(
                        out=pt[:], in_=curh[:, k * P : (k + 1) * P], identity=ident[:]
                    )
                    nc.vector.tensor_copy(out=curhT[:, k, :], in_=pt[:])

                ph = psum.tile([P, H], f32)
                for k in range(KC_IN):
                    nc.tensor.matmul(
                        out=ph[:],
                        lhsT=xT_sb[:, k, t * P : (t + 1) * P],
                        rhs=w_in_sb[:, k, :],
                        start=(k == 0),
                        stop=False,
                    )
                for k in range(KC_H):
                    nc.tensor.matmul(
                        out=ph[:],
                        lhsT=curhT[:, k, :],
                        rhs=w_h_sb[:, k, :],
                        start=False,
                        stop=(k == KC_H - 1),
                    )

                h_sb = work.tile([P, H], f32, tag=f"h{t}")
                nc.vector.tensor_add(out=h_sb[:], in0=ph[:], in1=vec_sb[:, 0, :])

                stats = work.tile([P, 6], f32)
                nc.vector.bn_stats(out=stats[:], in_=h_sb[:])
                mv = work.tile([P, 2], f32)
                nc.vector.bn_aggr(out=mv[:], in_=stats[:])
                # mv[:,1] = 1/sqrt(var + eps)
                nc.scalar.activation(
                    out=mv[:, 1:2],
                    in_=mv[:, 1:2],
                    func=mybir.ActivationFunctionType.Sqrt,
                    bias=eps_sb[:],
                    scale=1.0,
                )
                nc.vector.reciprocal(out=mv[:, 1:2], in_=mv[:, 1:2])
                # h = (h - mean) * rstd
                nc.vector.tensor_scalar(
                    out=h_sb[:],
                    in0=h_sb[:],
                    scalar1=mv[:, 0:1],
                    scalar2=mv[:, 1:2],
                    op0=mybir.AluOpType.subtract,
                    op1=mybir.AluOpType.mult,
                )
                nc.vector.tensor_mul(h_sb[:], h_sb[:], vec_sb[:, 1, :])
                nc.vector.tensor_add(out=h_sb[:], in0=h_sb[:], in1=vec_sb[:, 2, :])
                nc.scalar.activation(
                    out=h_sb[:],
                    in_=h_sb[:],
                    func=mybir.ActivationFunctionType.Tanh,
                )

                # Scatter row-tile t into its own output chunk. Indices are
                # rebased to the chunk on host (flat row r*L_V+idx - t*CHUNK).
                sc = nc.gpsimd.indirect_dma_start(
                    out=outs[t][:],
                    out_offset=bass.IndirectOffsetOnAxis(ap=idx_sbs[t][:, 1:2], axis=0),
                    in_=h_sb[:],
                    in_offset=None,
                )
                for ci in copy_insts[t]:
                    tile.add_dep_helper(
                        sc.ins, ci.ins, sync=True,
                        reason="scatter after bulk copy of its chunk",
                    )

    nc.finalize()
    return nc


def _prepare_in_maps(inputs: dict) -> list[dict]:
    memory = np.ascontiguousarray(np.asarray(inputs["memory"], dtype=np.float32))
    veh_idx = np.asarray(inputs["veh_idx"]).astype(np.int64)
    veh = np.asarray(inputs["veh_repr"], dtype=np.float32).reshape(N, D)
    cust = np.asarray(inputs["cust_repr"], dtype=np.float32).reshape(N, D)
    edge = np.asarray(inputs["edge_emb"], dtype=np.float32).reshape(N, D)
    w_in = np.ascontiguousarray(np.asarray(inputs["W_in"], dtype=np.float32))
    b_in = np.asarray(inputs["b_in"], dtype=np.float32)
    w_h = np.ascontiguousarray(np.asarray(inputs["W_h"], dtype=np.float32))
    b_h = np.asarray(inputs["b_h"], dtype=np.float32)
    gamma = np.asarray(inputs["gamma"], dtype=np.float32)
    beta = np.asarray(inputs["beta"], dtype=np.float32)

    x = np.concatenate([veh, cust, edge], axis=1)  # [N, 3D]
    vecs = np.ascontiguousarray(np.stack([b_in + b_h, gamma, beta]))  # [3, H]
    # flat row index within the core's [NS*L_V] space, then rebased per
    # 128-row tile chunk: row r of tile t scatters to chunk-local row
    # (r - t*P)*L_V + idx_r which equals flat - t*CHUNK.
    local_row = np.arange(N, dtype=np.int64) % NS
    gather_idx = (local_row * L_V + veh_idx[:, 0]).astype(np.int32)       # core space
    scatter_idx = (local_row % P * L_V + veh_idx[:, 0]).astype(np.int32)  # chunk space
    flat_idx = np.stack([gather_idx, scatter_idx], axis=1)                # [N, 2]

    in_maps = []
    for c in range(NCORES):
        rows = slice(c * NS, (c + 1) * NS)
        in_maps.append(
            {
                "mem": memory[rows].reshape(ROWS_FLAT, H),
                "xT": np.ascontiguousarray(x[rows].T),
                "idx": np.ascontiguousarray(flat_idx[rows].reshape(NS, 2)),
                "w_in": w_in,
                "w_h": w_h,
                "vecs": vecs,
            }
        )
    return in_maps


def get_nc() -> bass.Bass:
    if "nc" not in _CACHE:
        _CACHE["nc"] = _build_bass()
    return _CACHE["nc"]


def kernel(**inputs: np.ndarray) -> np.ndarray:
    nc = get_nc()
    in_maps = _prepare_in_maps(inputs)

    global LAST_RESULT
    LAST_RESULT = run_bass_kernel_spmd(nc, in_maps, list(range(NCORES)))
    res = LAST_RESULT.results
    return np.concatenate(
        [res[c][f"out{t}"] for c in range(NCORES) for t in range(RT)], axis=0
    ).reshape(N, L_V, H)
